# revision 4
# baseline (speedup 1.0000x reference)
"""Causal self-attention (B=2, L=2048, HID=2048, H=16, D=128) on 8 trn2 cores.

Sharding: core c -> (batch b = c//4, head-group g = c%4 of 4 heads).
Each core computes q/k/v projections for its 512 features from its batch,
RoPE, causal attention for its 4 heads, and a partial output projection
against its Wo column slice (fp16 partials). Host sums the 4 partials per
batch.

v2 structure (single pass over x per 512-column block):
  for ic: load x-block once -> V proj -> Q proj (+RoPE) -> K proj (+RoPE)
          -> Wo blocks of ic-1 (PE filler while RoPE drains on DVE/DMA)
          -> attention for query block ic (paired 2-bank exp activations).
RoPE rotate-half runs as two SBUF->SBUF partition-shift DMAs against a
sign-folded sin table (no PE matmul). All matmuls fp16 with fp32 PSUM.
Softmax skips max-subtraction (exp gets a -4 bias that cancels in the
normalization); denominator via an all-ones stationary matmul.
"""
import numpy as np

import concourse.mybir as mybir
import concourse.tile as tile
from concourse import bacc
from concourse.bass_utils import run_bass_kernel_spmd

B, L, HID, H = 2, 2048, 2048, 16
D = 128               # head dim
NCORES = 8
GH = 4                # heads per core
E = GH * D            # 512 per-core qkv features
NT = HID // 128       # 16 contraction tiles
NI = L // 512         # 4 i-chunks of 512
SCALE = 1.0 / float(np.sqrt(D))

F32 = mybir.dt.float32
MULT = mybir.AluOpType.mult
ADD = mybir.AluOpType.add
IS_GE = mybir.AluOpType.is_ge
DT = mybir.dt.float16       # on-chip matmul dtype
NP_DT = np.float16
EXP_BIAS = -4.0             # exp(s*scale - 4): fp16 overflow headroom, cancels in softmax


def _emit(nc, tc, ctx, io):
    xT, wqT, wkT, wvT, woT, cosT, sinT, out = (
        io["xT"], io["wqT"], io["wkT"], io["wvT"], io["woT"],
        io["cosT"], io["sinT"], io["out"],
    )
    xTr = xT.rearrange("(t p) i -> p t i", p=128)       # [128, 16, 2048]
    wqTr = wqT.rearrange("(t p) e -> p t e", p=128)     # [128, 16, 512]
    wkTr = wkT.rearrange("(t p) e -> p t e", p=128)
    wvTr = wvT.rearrange("(t p) e -> p t e", p=128)
    woTr = woT.rearrange("(s p) f -> p s f", p=128)     # [128, 4, 2048]

    pool = ctx.enter_context(tc.tile_pool(name="main", bufs=1))
    xpool = ctx.enter_context(tc.tile_pool(name="xf", bufs=2))
    work = ctx.enter_context(tc.tile_pool(name="work", bufs=2))
    obpool = ctx.enter_context(tc.tile_pool(name="ob", bufs=3))
    dpool = ctx.enter_context(tc.tile_pool(name="dp", bufs=2))
    # PSUM: mm(2x1) + stb(2x2) + ov(1) + dn(1) = 8 banks
    ps = ctx.enter_context(tc.tile_pool(name="ps", bufs=1, space="PSUM"))

    # ---- persistent SBUF tiles ----
    wv_sb = pool.tile([128, NT, 512], DT, tag="wv")
    wq_sb = pool.tile([128, NT, 512], DT, tag="wq")
    wk_sb = pool.tile([128, NT, 512], DT, tag="wk")
    wo_sb = pool.tile([128, GH, L], DT, tag="wo")
    cos_sb = pool.tile([128, L], DT, tag="cos")
    sin_sb = pool.tile([128, L], DT, tag="sin")   # sign-folded: rows 0:64 negated
    ebias = pool.tile([128, 1], F32, tag="ebias")
    ones = pool.tile([128, 128], DT, tag="ones")
    v_sb = [pool.tile([128, E], DT, tag=f"v{jt}", name=f"v{jt}") for jt in range(NT)]
    qr = [pool.tile([128, L], DT, tag=f"qr{h}", name=f"qr{h}") for h in range(GH)]
    kr = [pool.tile([128, L], DT, tag=f"kr{h}", name=f"kr{h}") for h in range(GH)]
    ot = [pool.tile([128, L], DT, tag=f"ot{h}", name=f"ot{h}") for h in range(GH)]
    # exp output pairs: ep[jp] covers key tiles (2jp, 2jp+1)
    ep = [pool.tile([128, 1024], DT, tag=f"E{jp}", name=f"ep{jp}") for jp in range(NT // 2)]

    nc.gpsimd.memset(ebias[:], EXP_BIAS)
    nc.gpsimd.memset(ones[:], 1.0)

    xf_tiles = [xpool.tile([128, NT, 512], DT, tag="xf", name=f"xf{ic}") for ic in range(NI)]

    # ---- prologue DMAs: finest-grain first so matmul 0 starts early ----
    xf0 = xf_tiles[0]
    for mt in range(NT):
        nc.sync.dma_start(xf0[:, mt : mt + 1, :], xTr[:, mt : mt + 1, 0:512])
        nc.sync.dma_start(wv_sb[:, mt : mt + 1, :], wvTr[:, mt : mt + 1, :])
    for c in range(4):
        nc.sync.dma_start(wq_sb[:, 4 * c : 4 * c + 4, :], wqTr[:, 4 * c : 4 * c + 4, :])
    for c in range(4):
        nc.sync.dma_start(wk_sb[:, 4 * c : 4 * c + 4, :], wkTr[:, 4 * c : 4 * c + 4, :])
    nc.sync.dma_start(cos_sb[:], cosT)
    nc.sync.dma_start(sin_sb[:], sinT)
    for s_ in range(GH):
        nc.sync.dma_start(wo_sb[:, s_, :], woTr[:, s_, :])

    def emit_rope(pps, dst, dt, isl):
        """dst[dt][:, isl] = pre*cos + rot_half(pre)*sin, via partition-shift DMA."""
        pre = work.tile([128, 512], DT, tag="pre", bufs=4)
        nc.scalar.copy(pre[:], pps[:])
        shuf = work.tile([128, 512], DT, tag="shuf", bufs=4)
        nc.sync.dma_start(shuf[0:64, :], pre[64:128, :])
        nc.sync.dma_start(shuf[64:128, :], pre[0:64, :])
        t1 = work.tile([128, 512], DT, tag="t1", bufs=2)
        nc.vector.tensor_tensor(t1[:], pre[:], cos_sb[:, isl], MULT)
        t2 = work.tile([128, 512], DT, tag="t2", bufs=2)
        nc.vector.tensor_tensor(t2[:], shuf[:], sin_sb[:, isl], MULT)
        nc.gpsimd.tensor_tensor(dst[dt][:, isl], t1[:], t2[:], ADD)

    def emit_wo_block(I):
        """Partial output projection for query tiles of block I (16 [128,512] units)."""
        for it in range(I * 4, I * 4 + 4):
            for fp in range(2):
                ob = obpool.tile([128, 1024], DT, tag="ob", name="ob")
                for half in range(2):
                    fc = 2 * fp + half
                    op = ps.tile([128, 512], F32, tag="mm", bufs=2, name="op")
                    for h in range(GH):
                        nc.tensor.matmul(
                            op[:],
                            ot[h][:, it * 128 : (it + 1) * 128],
                            wo_sb[:, h, fc * 512 : (fc + 1) * 512],
                            start=(h == 0),
                            stop=(h == GH - 1),
                        )
                    if (it + fc) % 2 == 0:
                        nc.vector.tensor_copy(ob[:, half * 512 : (half + 1) * 512], op[:])
                    else:
                        nc.scalar.copy(ob[:, half * 512 : (half + 1) * 512], op[:])
                nc.sync.dma_start(
                    out[it * 128 : (it + 1) * 128, fp * 1024 : (fp + 1) * 1024], ob[:]
                )

    for ic in range(NI):
        xf = xf_tiles[ic]
        isl = slice(ic * 512, (ic + 1) * 512)
        if ic > 0:
            for g in range(4):
                nc.sync.dma_start(
                    xf[:, 4 * g : 4 * g + 4, :], xTr[:, 4 * g : 4 * g + 4, isl]
                )

        # ---- V projection for this block -> v_sb[4ic..4ic+3] ----
        if ic == 0:
            # mt-interleaved so compute paces with the prologue DMA stream
            vps = [
                ps.tile([128, 512], F32, tag=("mm" if jt < 2 else "stb"), bufs=2,
                        name=f"vp{jt}")
                for jt in range(4)
            ]
            for mt in range(NT):
                for jt in range(4):
                    nc.tensor.matmul(
                        vps[jt][:],
                        xf[:, mt, jt * 128 : (jt + 1) * 128],
                        wv_sb[:, mt, :],
                        start=(mt == 0),
                        stop=(mt == NT - 1),
                    )
            for jt in range(4):
                if jt % 2 == 0:
                    nc.scalar.copy(v_sb[jt][:], vps[jt][:])
                else:
                    nc.vector.tensor_copy(v_sb[jt][:], vps[jt][:])
        else:
            # group-major: each group's copy overlaps the next group's matmuls
            for jt in range(4):
                vps = ps.tile([128, 512], F32, tag="mm", bufs=2, name="vp")
                for mt in range(NT):
                    nc.tensor.matmul(
                        vps[:],
                        xf[:, mt, jt * 128 : (jt + 1) * 128],
                        wv_sb[:, mt, :],
                        start=(mt == 0),
                        stop=(mt == NT - 1),
                    )
                if jt % 2 == 0:
                    nc.scalar.copy(v_sb[4 * ic + jt][:], vps[:])
                else:
                    nc.vector.tensor_copy(v_sb[4 * ic + jt][:], vps[:])

        # ---- Q then K projection (+RoPE) for this block ----
        for w_sb, dst in ((wq_sb, qr), (wk_sb, kr)):
            for dt in range(GH):
                pps = ps.tile([128, 512], F32, tag="mm", bufs=2, name="pp")
                for mt in range(NT):
                    nc.tensor.matmul(
                        pps[:],
                        w_sb[:, mt, dt * 128 : (dt + 1) * 128],
                        xf[:, mt, :],
                        start=(mt == 0),
                        stop=(mt == NT - 1),
                    )
                emit_rope(pps, dst, dt, isl)

        # ---- Wo blocks for the previous query block: PE filler while RoPE drains
        if ic > 0:
            emit_wo_block(ic - 1)

        # ---- attention for query block I = ic, all heads ----
        I = ic
        nj = (I + 1) * 4
        npair = nj // 2
        i0 = I * 512

        def vc0(jt):
            # diag tile jt = I*4 + t has valid columns [128*t, 512) only
            return max(0, (jt - I * 4) * 128)

        for h in range(GH):
            for jp in range(npair):
                stb = ps.tile([128, 1024], F32, tag="stb", bufs=2, name="stb")
                for t in range(2):
                    jt = 2 * jp + t
                    c0 = vc0(jt)
                    nc.tensor.matmul(
                        stb[:, 512 * t + c0 : 512 * (t + 1)],
                        kr[h][:, jt * 128 : (jt + 1) * 128],
                        qr[h][:, i0 + c0 : i0 + 512],
                        start=True,
                        stop=True,
                    )
                ex = ep[jp]
                nc.scalar.activation(
                    ex[:], stb[:], mybir.ActivationFunctionType.Exp,
                    scale=SCALE, bias=ebias[:],
                )
                for t in range(2):
                    jt = 2 * jp + t
                    if jt >= I * 4:
                        c0 = vc0(jt)
                        # within valid cols keep upper triangle: c' - p >= 0
                        nc.gpsimd.affine_select(
                            out=ex[:, 512 * t + c0 : 512 * (t + 1)],
                            in_=ex[:, 512 * t + c0 : 512 * (t + 1)],
                            compare_op=IS_GE,
                            fill=0.0,
                            base=0,
                            pattern=[[1, 512 - c0]],
                            channel_multiplier=-1,
                        )
            ov = ps.tile([128, 512], F32, tag="ov", bufs=1, name="ov")
            dn = ps.tile([128, 512], F32, tag="dn", bufs=1, name="dn")
            for jp in range(npair):
                # DN first so the reciprocal overlaps AV's tail matmuls
                for t in range(2):
                    jt = 2 * jp + t
                    c0 = vc0(jt)
                    nc.tensor.matmul(
                        dn[:, c0:],
                        ones[:],
                        ep[jp][:, 512 * t + c0 : 512 * (t + 1)],
                        start=(jt == 0),
                        stop=(jt == nj - 1),
                    )
                for t in range(2):
                    jt = 2 * jp + t
                    c0 = vc0(jt)
                    nc.tensor.matmul(
                        ov[:, c0:],
                        v_sb[jt][:, h * 128 : (h + 1) * 128],
                        ep[jp][:, 512 * t + c0 : 512 * (t + 1)],
                        start=(jt == 0),
                        stop=(jt == nj - 1),
                    )
            rbi = dpool.tile([128, 512], F32, tag="rbi", bufs=2)
            nc.vector.reciprocal_approx_fast(out=rbi[:], in_=dn[:])
            nc.vector.tensor_tensor(ot[h][:, i0 : i0 + 512], ov[:], rbi[:], MULT)

    emit_wo_block(NI - 1)


def build():
    import contextlib

    nc = bacc.Bacc("TRN2", target_bir_lowering=False, debug=False, num_devices=NCORES)
    io = {
        "xT": nc.dram_tensor("xT", [HID, L], DT, kind="ExternalInput").ap(),
        "wqT": nc.dram_tensor("wqT", [HID, E], DT, kind="ExternalInput").ap(),
        "wkT": nc.dram_tensor("wkT", [HID, E], DT, kind="ExternalInput").ap(),
        "wvT": nc.dram_tensor("wvT", [HID, E], DT, kind="ExternalInput").ap(),
        "woT": nc.dram_tensor("woT", [E, HID], DT, kind="ExternalInput").ap(),
        "cosT": nc.dram_tensor("cosT", [D, L], DT, kind="ExternalInput").ap(),
        "sinT": nc.dram_tensor("sinT", [D, L], DT, kind="ExternalInput").ap(),
        "out": nc.dram_tensor("out", [L, HID], DT, kind="ExternalOutput").ap(),
    }
    with tile.TileContext(nc) as tc:
        with contextlib.ExitStack() as ctx:
            _emit(nc, tc, ctx, io)
    nc.compile()
    return nc


_NC_CACHE = []


def make_in_maps(hidden_states, cos, sin, Wq, Wk, Wv, Wo):
    f = NP_DT
    cosT = np.ascontiguousarray(cos.T.astype(f))
    sin_pm = sin.T.astype(np.float32).copy()
    sin_pm[0 : D // 2] = -sin_pm[0 : D // 2]
    sinT = np.ascontiguousarray(sin_pm.astype(f))
    xTs = [np.ascontiguousarray(hidden_states[b].T.astype(f)) for b in range(B)]
    in_maps = []
    for c in range(NCORES):
        b, g = divmod(c, 4)
        sl = slice(g * E, (g + 1) * E)
        in_maps.append({
            "xT": xTs[b],
            "wqT": np.ascontiguousarray(Wq[sl].T.astype(f)),
            "wkT": np.ascontiguousarray(Wk[sl].T.astype(f)),
            "wvT": np.ascontiguousarray(Wv[sl].T.astype(f)),
            "woT": np.ascontiguousarray(Wo[:, sl].T.astype(f)),
            "cosT": cosT,
            "sinT": sinT,
        })
    return in_maps


def kernel(hidden_states, cos, sin, Wq, Wk, Wv, Wo):
    hidden_states, cos, sin, Wq, Wk, Wv, Wo = (
        np.asarray(a) for a in (hidden_states, cos, sin, Wq, Wk, Wv, Wo)
    )
    if not _NC_CACHE:
        _NC_CACHE.append(build())
    nc = _NC_CACHE[0]
    in_maps = make_in_maps(hidden_states, cos, sin, Wq, Wk, Wv, Wo)
    r = run_bass_kernel_spmd(nc, in_maps, list(range(NCORES)))
    out = np.empty((B, L, HID), np.float32)
    for b in range(B):
        acc = r.results[4 * b]["out"].astype(np.float32)
        for g in range(1, 4):
            acc = acc + r.results[4 * b + g]["out"].astype(np.float32)
        out[b] = acc
    return out


# revision 5
# speedup vs baseline: 1.0214x; 1.0214x over previous
"""Causal self-attention (B=2, L=2048, HID=2048, H=16, D=128) on 8 trn2 cores.

Sharding: core c -> (batch b = c//4, head-group g = c%4 of 4 heads).
Each core computes q/k/v projections for its 512 features from its batch,
RoPE, causal attention for its 4 heads, and a partial output projection
against its Wo column slice (fp16 partials). Host sums the 4 partials per
batch.

v2 structure (single pass over x per 512-column block):
  for ic: load x-block once -> V proj -> Q proj (+RoPE) -> K proj (+RoPE)
          -> Wo blocks of ic-1 (PE filler while RoPE drains on DVE/DMA)
          -> attention for query block ic (paired 2-bank exp activations).
RoPE rotate-half runs as two SBUF->SBUF partition-shift DMAs against a
sign-folded sin table (no PE matmul). All matmuls fp16 with fp32 PSUM.
Softmax skips max-subtraction (exp gets a -4 bias that cancels in the
normalization); denominator via an all-ones stationary matmul.
"""
import numpy as np

import concourse.mybir as mybir
import concourse.tile as tile
from concourse import bacc
from concourse.bass_utils import run_bass_kernel_spmd

B, L, HID, H = 2, 2048, 2048, 16
D = 128               # head dim
NCORES = 8
GH = 4                # heads per core
E = GH * D            # 512 per-core qkv features
NT = HID // 128       # 16 contraction tiles
NI = L // 512         # 4 i-chunks of 512
SCALE = 1.0 / float(np.sqrt(D))

F32 = mybir.dt.float32
MULT = mybir.AluOpType.mult
ADD = mybir.AluOpType.add
IS_GE = mybir.AluOpType.is_ge
DT = mybir.dt.float16       # on-chip matmul dtype
NP_DT = np.float16
EXP_BIAS = -4.0             # exp(s*scale - 4): fp16 overflow headroom, cancels in softmax


def _emit(nc, tc, ctx, io):
    xT, wqT, wkT, wvT, woT, cosT, sinT, out = (
        io["xT"], io["wqT"], io["wkT"], io["wvT"], io["woT"],
        io["cosT"], io["sinT"], io["out"],
    )
    xTr = xT.rearrange("(t p) i -> p t i", p=128)       # [128, 16, 2048]
    wqTr = wqT.rearrange("(t p) e -> p t e", p=128)     # [128, 16, 512]
    wkTr = wkT.rearrange("(t p) e -> p t e", p=128)
    wvTr = wvT.rearrange("(t p) e -> p t e", p=128)
    woTr = woT.rearrange("(s p) f -> p s f", p=128)     # [128, 4, 2048]

    pool = ctx.enter_context(tc.tile_pool(name="main", bufs=1))
    xpool = ctx.enter_context(tc.tile_pool(name="xf", bufs=2))
    work = ctx.enter_context(tc.tile_pool(name="work", bufs=2))
    obpool = ctx.enter_context(tc.tile_pool(name="ob", bufs=3))
    dpool = ctx.enter_context(tc.tile_pool(name="dp", bufs=2))
    # PSUM: mm(2x1) + stb(2x2) + ov(1) + dn(1) = 8 banks
    ps = ctx.enter_context(tc.tile_pool(name="ps", bufs=1, space="PSUM"))

    # ---- persistent SBUF tiles ----
    wv_sb = pool.tile([128, NT, 512], DT, tag="wv")
    wq_sb = pool.tile([128, NT, 512], DT, tag="wq")
    wk_sb = pool.tile([128, NT, 512], DT, tag="wk")
    wo_sb = pool.tile([128, GH, L], DT, tag="wo")
    cos_sb = pool.tile([128, L], DT, tag="cos")
    sin_sb = pool.tile([128, L], DT, tag="sin")   # sign-folded: rows 0:64 negated
    ebias = pool.tile([128, 1], F32, tag="ebias")
    ones = pool.tile([128, 128], DT, tag="ones")
    v_sb = [pool.tile([128, E], DT, tag=f"v{jt}", name=f"v{jt}") for jt in range(NT)]
    qr = [pool.tile([128, L], DT, tag=f"qr{h}", name=f"qr{h}") for h in range(GH)]
    kr = [pool.tile([128, L], DT, tag=f"kr{h}", name=f"kr{h}") for h in range(GH)]
    ot = [pool.tile([128, L], DT, tag=f"ot{h}", name=f"ot{h}") for h in range(GH)]
    # exp output pairs: ep[jp] covers key tiles (2jp, 2jp+1)
    ep = [pool.tile([128, 1024], DT, tag=f"E{jp}", name=f"ep{jp}") for jp in range(NT // 2)]

    nc.gpsimd.memset(ebias[:], EXP_BIAS)
    nc.gpsimd.memset(ones[:], 1.0)

    xf_tiles = [xpool.tile([128, NT, 512], DT, tag="xf", name=f"xf{ic}") for ic in range(NI)]

    # ---- prologue DMAs: finest-grain first so matmul 0 starts early ----
    xf0 = xf_tiles[0]
    nc.sync.dma_start(xf0[:, 0:1, :], xTr[:, 0:1, 0:512])
    nc.sync.dma_start(wv_sb[:, 0:1, :], wvTr[:, 0:1, :])
    nc.sync.dma_start(xf0[:, 1:4, :], xTr[:, 1:4, 0:512])
    nc.sync.dma_start(wv_sb[:, 1:4, :], wvTr[:, 1:4, :])
    for g in range(1, 4):
        nc.sync.dma_start(xf0[:, 4 * g : 4 * g + 4, :], xTr[:, 4 * g : 4 * g + 4, 0:512])
        nc.sync.dma_start(wv_sb[:, 4 * g : 4 * g + 4, :], wvTr[:, 4 * g : 4 * g + 4, :])
    for c in range(4):
        nc.sync.dma_start(wq_sb[:, 4 * c : 4 * c + 4, :], wqTr[:, 4 * c : 4 * c + 4, :])
    for c in range(4):
        nc.sync.dma_start(wk_sb[:, 4 * c : 4 * c + 4, :], wkTr[:, 4 * c : 4 * c + 4, :])
    nc.sync.dma_start(cos_sb[:], cosT)
    nc.sync.dma_start(sin_sb[:], sinT)
    for s_ in range(GH):
        nc.sync.dma_start(wo_sb[:, s_, :], woTr[:, s_, :])

    def emit_rope(pps, dst, dt, isl):
        """dst[dt][:, isl] = pre*cos + rot_half(pre)*sin, via partition-shift DMA."""
        pre = work.tile([128, 512], DT, tag="pre", bufs=4)
        nc.scalar.copy(pre[:], pps[:])
        shuf = work.tile([128, 512], DT, tag="shuf", bufs=4)
        nc.sync.dma_start(shuf[0:64, :], pre[64:128, :])
        nc.sync.dma_start(shuf[64:128, :], pre[0:64, :])
        t1 = work.tile([128, 512], DT, tag="t1", bufs=2)
        nc.vector.tensor_tensor(t1[:], pre[:], cos_sb[:, isl], MULT)
        t2 = work.tile([128, 512], DT, tag="t2", bufs=2)
        nc.vector.tensor_tensor(t2[:], shuf[:], sin_sb[:, isl], MULT)
        nc.gpsimd.tensor_tensor(dst[dt][:, isl], t1[:], t2[:], ADD)

    def emit_wo_block(I):
        """Partial output projection for query tiles of block I (16 [128,512] units)."""
        for it in range(I * 4, I * 4 + 4):
            for fp in range(2):
                ob = obpool.tile([128, 1024], DT, tag="ob", name="ob")
                for half in range(2):
                    fc = 2 * fp + half
                    op = ps.tile([128, 512], F32, tag="mm", bufs=2, name="op")
                    for h in range(GH):
                        nc.tensor.matmul(
                            op[:],
                            ot[h][:, it * 128 : (it + 1) * 128],
                            wo_sb[:, h, fc * 512 : (fc + 1) * 512],
                            start=(h == 0),
                            stop=(h == GH - 1),
                        )
                    if (it + fc) % 2 == 0:
                        nc.vector.tensor_copy(ob[:, half * 512 : (half + 1) * 512], op[:])
                    else:
                        nc.scalar.copy(ob[:, half * 512 : (half + 1) * 512], op[:])
                nc.sync.dma_start(
                    out[it * 128 : (it + 1) * 128, fp * 1024 : (fp + 1) * 1024], ob[:]
                )

    for ic in range(NI):
        xf = xf_tiles[ic]
        isl = slice(ic * 512, (ic + 1) * 512)
        if ic > 0:
            for g in range(4):
                nc.sync.dma_start(
                    xf[:, 4 * g : 4 * g + 4, :], xTr[:, 4 * g : 4 * g + 4, isl]
                )

        # ---- V projection for this block -> v_sb[4ic..4ic+3] ----
        if ic == 0:
            # mt-interleaved so compute paces with the prologue DMA stream
            vps = [
                ps.tile([128, 512], F32, tag=("mm" if jt < 2 else "stb"), bufs=2,
                        name=f"vp{jt}")
                for jt in range(4)
            ]
            for mt in range(NT):
                for jt in range(4):
                    nc.tensor.matmul(
                        vps[jt][:],
                        xf[:, mt, jt * 128 : (jt + 1) * 128],
                        wv_sb[:, mt, :],
                        start=(mt == 0),
                        stop=(mt == NT - 1),
                    )
            for jt in range(4):
                if jt % 2 == 0:
                    nc.scalar.copy(v_sb[jt][:], vps[jt][:])
                else:
                    nc.vector.tensor_copy(v_sb[jt][:], vps[jt][:])
        else:
            # group-major: each group's copy overlaps the next group's matmuls
            for jt in range(4):
                vps = ps.tile([128, 512], F32, tag="mm", bufs=2, name="vp")
                for mt in range(NT):
                    nc.tensor.matmul(
                        vps[:],
                        xf[:, mt, jt * 128 : (jt + 1) * 128],
                        wv_sb[:, mt, :],
                        start=(mt == 0),
                        stop=(mt == NT - 1),
                    )
                if jt % 2 == 0:
                    nc.scalar.copy(v_sb[4 * ic + jt][:], vps[:])
                else:
                    nc.vector.tensor_copy(v_sb[4 * ic + jt][:], vps[:])

        # ---- Q then K projection (+RoPE) for this block ----
        for w_sb, dst in ((wq_sb, qr), (wk_sb, kr)):
            for dt in range(GH):
                pps = ps.tile([128, 512], F32, tag="mm", bufs=2, name="pp")
                for mt in range(NT):
                    nc.tensor.matmul(
                        pps[:],
                        w_sb[:, mt, dt * 128 : (dt + 1) * 128],
                        xf[:, mt, :],
                        start=(mt == 0),
                        stop=(mt == NT - 1),
                    )
                emit_rope(pps, dst, dt, isl)

        # ---- Wo blocks for the previous query block: PE filler while RoPE drains
        if ic > 0:
            emit_wo_block(ic - 1)

        # ---- attention for query block I = ic, all heads ----
        I = ic
        nj = (I + 1) * 4
        npair = nj // 2
        i0 = I * 512

        def vc0(jt):
            # diag tile jt = I*4 + t has valid columns [128*t, 512) only
            return max(0, (jt - I * 4) * 128)

        for h in range(GH):
            for jp in range(npair):
                stb = ps.tile([128, 1024], F32, tag="stb", bufs=2, name="stb")
                for t in range(2):
                    jt = 2 * jp + t
                    c0 = vc0(jt)
                    nc.tensor.matmul(
                        stb[:, 512 * t + c0 : 512 * (t + 1)],
                        kr[h][:, jt * 128 : (jt + 1) * 128],
                        qr[h][:, i0 + c0 : i0 + 512],
                        start=True,
                        stop=True,
                    )
                ex = ep[jp]
                nc.scalar.activation(
                    ex[:], stb[:], mybir.ActivationFunctionType.Exp,
                    scale=SCALE, bias=ebias[:],
                )
                for t in range(2):
                    jt = 2 * jp + t
                    if jt >= I * 4:
                        c0 = vc0(jt)
                        # within valid cols keep upper triangle: c' - p >= 0
                        nc.gpsimd.affine_select(
                            out=ex[:, 512 * t + c0 : 512 * (t + 1)],
                            in_=ex[:, 512 * t + c0 : 512 * (t + 1)],
                            compare_op=IS_GE,
                            fill=0.0,
                            base=0,
                            pattern=[[1, 512 - c0]],
                            channel_multiplier=-1,
                        )
            ov = ps.tile([128, 512], F32, tag="ov", bufs=1, name="ov")
            dn = ps.tile([128, 512], F32, tag="dn", bufs=1, name="dn")
            for jp in range(npair):
                # DN first so the reciprocal overlaps AV's tail matmuls
                for t in range(2):
                    jt = 2 * jp + t
                    c0 = vc0(jt)
                    nc.tensor.matmul(
                        dn[:, c0:],
                        ones[:],
                        ep[jp][:, 512 * t + c0 : 512 * (t + 1)],
                        start=(jt == 0),
                        stop=(jt == nj - 1),
                    )
                for t in range(2):
                    jt = 2 * jp + t
                    c0 = vc0(jt)
                    nc.tensor.matmul(
                        ov[:, c0:],
                        v_sb[jt][:, h * 128 : (h + 1) * 128],
                        ep[jp][:, 512 * t + c0 : 512 * (t + 1)],
                        start=(jt == 0),
                        stop=(jt == nj - 1),
                    )
            rbi = dpool.tile([128, 512], F32, tag="rbi", bufs=2)
            nc.vector.reciprocal_approx_fast(out=rbi[:], in_=dn[:])
            nc.vector.tensor_tensor(ot[h][:, i0 : i0 + 512], ov[:], rbi[:], MULT)

    emit_wo_block(NI - 1)


def build():
    import contextlib

    nc = bacc.Bacc("TRN2", target_bir_lowering=False, debug=False, num_devices=NCORES)
    io = {
        "xT": nc.dram_tensor("xT", [HID, L], DT, kind="ExternalInput").ap(),
        "wqT": nc.dram_tensor("wqT", [HID, E], DT, kind="ExternalInput").ap(),
        "wkT": nc.dram_tensor("wkT", [HID, E], DT, kind="ExternalInput").ap(),
        "wvT": nc.dram_tensor("wvT", [HID, E], DT, kind="ExternalInput").ap(),
        "woT": nc.dram_tensor("woT", [E, HID], DT, kind="ExternalInput").ap(),
        "cosT": nc.dram_tensor("cosT", [D, L], DT, kind="ExternalInput").ap(),
        "sinT": nc.dram_tensor("sinT", [D, L], DT, kind="ExternalInput").ap(),
        "out": nc.dram_tensor("out", [L, HID], DT, kind="ExternalOutput").ap(),
    }
    with tile.TileContext(nc) as tc:
        with contextlib.ExitStack() as ctx:
            _emit(nc, tc, ctx, io)
    nc.compile()
    return nc


_NC_CACHE = []


def make_in_maps(hidden_states, cos, sin, Wq, Wk, Wv, Wo):
    f = NP_DT
    cosT = np.ascontiguousarray(cos.T.astype(f))
    sin_pm = sin.T.astype(np.float32).copy()
    sin_pm[0 : D // 2] = -sin_pm[0 : D // 2]
    sinT = np.ascontiguousarray(sin_pm.astype(f))
    xTs = [np.ascontiguousarray(hidden_states[b].T.astype(f)) for b in range(B)]
    in_maps = []
    for c in range(NCORES):
        b, g = divmod(c, 4)
        sl = slice(g * E, (g + 1) * E)
        in_maps.append({
            "xT": xTs[b],
            "wqT": np.ascontiguousarray(Wq[sl].T.astype(f)),
            "wkT": np.ascontiguousarray(Wk[sl].T.astype(f)),
            "wvT": np.ascontiguousarray(Wv[sl].T.astype(f)),
            "woT": np.ascontiguousarray(Wo[:, sl].T.astype(f)),
            "cosT": cosT,
            "sinT": sinT,
        })
    return in_maps


def kernel(hidden_states, cos, sin, Wq, Wk, Wv, Wo):
    hidden_states, cos, sin, Wq, Wk, Wv, Wo = (
        np.asarray(a) for a in (hidden_states, cos, sin, Wq, Wk, Wv, Wo)
    )
    if not _NC_CACHE:
        _NC_CACHE.append(build())
    nc = _NC_CACHE[0]
    in_maps = make_in_maps(hidden_states, cos, sin, Wq, Wk, Wv, Wo)
    r = run_bass_kernel_spmd(nc, in_maps, list(range(NCORES)))
    out = np.empty((B, L, HID), np.float32)
    for b in range(B):
        acc = r.results[4 * b]["out"].astype(np.float32)
        for g in range(1, 4):
            acc = acc + r.results[4 * b + g]["out"].astype(np.float32)
        out[b] = acc
    return out
